# revision 1
# baseline (speedup 1.0000x reference)
"""Causal self-attention, tensor-parallel over heads across 8 NeuronCores.

Reference:  qkv = x @ w_qkv + b_qkv;  per-head causal softmax attention;
            out = y @ w_proj + b_proj.
Shapes: x [2, 2048, 1024], H=16 heads, head_dim 64.

Sharding (per core c of 8): heads {2c, 2c+1}.
  - w_qkv columns for q/k/v of those heads  -> [1024, 384]
  - w_proj rows for those heads             -> [128, 1024]
  - each core computes a partial projection output; host sums the 8
    partials (the "all-reduce after proj") and adds nothing else
    (b_proj is fed to core 0 only).

On-device layout strategy (all matmuls fp32r = full-rate, ~1.6e-4 rel err):
  - host passes x transposed (xT [1024, 4096]) so features sit on the
    partition axis; QKV is computed directly in transposed form
    qkv_T [f, t], which is exactly the layout scores need.
  - scores are computed transposed: s_T[k, q] = K Q^T per head, softmax
    along the partition (k) axis with no max-subtraction (scores are
    ~N(0,1) for these inputs; exp is safe in fp32).
  - sum_k exp(s) is obtained for free by appending a ones-column to V
    (stationary operand of the P~@V matmul).
  - causal masking: fully-masked k-tiles are skipped outright; the 4
    diagonal k-tiles per q-chunk are masked with precomputed 0/1 tiles.
  - normalization (divide by sumexp) happens on the 64-row o_T tile via
    a PE-broadcast reciprocal row.
  - projection consumes y_T directly as the stationary operand.
"""

import numpy as np

import concourse.bacc as bacc
import concourse.mybir as mybir
import concourse.tile as tile
from concourse import bass_utils
from concourse import masks as cmasks

# Problem shapes (hardcoded per contest contract)
B, T, D = 2, 2048, 1024
H, HD = 16, 64
N_CORES = 8
HLOC = H // N_CORES      # 2 heads per core
FQ = HLOC * HD           # 128 features per core per q/k/v
BT = B * T               # 4096
TQ = 512                 # q-chunk (matmul moving dim)
KT = 128                 # k-tile (partition dim of transposed scores)
NQC = T // TQ            # 4 q-chunks per batch
NKT = T // KT            # 16 k-tiles per batch
NCT = D // 128           # 8 contraction tiles for qkv

F32 = mybir.dt.float32
F32R = mybir.dt.float32r
EXP = mybir.ActivationFunctionType.Exp


def build_nc(reps=1):
    nc = bacc.Bacc("TRN2", debug=False)

    xT = nc.dram_tensor("xT", (D, BT), F32R, kind="ExternalInput")
    wqkv = nc.dram_tensor("wqkv", (D, 3 * FQ), F32R, kind="ExternalInput")
    bqkv = nc.dram_tensor("bqkv", (128, 3), F32, kind="ExternalInput")
    wproj = nc.dram_tensor("wproj", (FQ, D), F32R, kind="ExternalInput")
    tri_d = nc.dram_tensor("tri", (128, 256), F32R, kind="ExternalInput")
    ones_d = nc.dram_tensor("ones_row", (1, 128), F32R, kind="ExternalInput")
    onesc_d = nc.dram_tensor("ones_col", (128, 1), F32R, kind="ExternalInput")
    out = nc.dram_tensor("out", (BT, D), F32, kind="ExternalOutput")

    xT_r = xT.rearrange("(ct p) t -> p ct t", p=128)
    wq_r = wqkv.rearrange("(ct p) (f m) -> p f ct m", p=128, f=3)

    with tile.TileContext(nc) as tc:
        with (
            tc.tile_pool(name="const", bufs=1) as cpool,
            tc.tile_pool(name="xt", bufs=3) as xpool,
            tc.tile_pool(name="vsb", bufs=2) as vpool,
            tc.tile_pool(name="pp", bufs=20) as ppool,
            tc.tile_pool(name="sm", bufs=2) as spool,
            tc.tile_pool(name="osb", bufs=6) as opool,
            tc.tile_pool(name="ps", bufs=1, space="PSUM") as ps,
        ):
            # ---- persistent tiles (tiny + first-needed DMAs first) ----
            onesr = cpool.tile([1, 128], F32R)
            nc.sync.dma_start(onesr[:], ones_d[:])
            onesc = cpool.tile([128, 1], F32R)
            nc.sync.dma_start(onesc[:], onesc_d[:])
            bsb = cpool.tile([128, 3], F32)
            nc.sync.dma_start(bsb[:], bqkv[:])
            wsb = cpool.tile([128, 3, NCT, 128], F32R)
            nc.sync.dma_start(wsb[:, 0, 0:NCT // 2], wq_r[:, 0, 0:NCT // 2])
            nc.sync.dma_start(wsb[:, 0, NCT // 2:], wq_r[:, 0, NCT // 2:])
            wpsb = cpool.tile([128, D], F32R)
            trisb = cpool.tile([128, 256], F32R)
            ident = cpool.tile([128, 128], F32)
            cmasks.make_identity(nc, ident[:])

            qT = cpool.tile([128, BT], F32R)
            kTt = cpool.tile([128, BT], F32R)
            yT = cpool.tile([128, BT], F32R)
            Vp = cpool.tile([128, HLOC, B * NKT, HD + 1], F32R)

            pend = None

            # V' ones-columns written once (persistent; V data cols are
            # rewritten per batch, col 64 never changes)
            for h in range(HLOC):
                for i in range(B * NKT):
                    nc.vector.tensor_copy(Vp[:, h, i, HD:HD + 1], onesc[:])


            def emit_proj(qoff_abs):
                for tt in range(TQ // 128):
                    toff = qoff_abs + tt * 128
                    for e in range(D // TQ):
                        ppj = ps.tile([128, TQ], F32, tag="s", bufs=5)
                        nc.tensor.matmul(
                            ppj[:], yT[:, toff:toff + 128],
                            wpsb[:, e * TQ:(e + 1) * TQ],
                            start=True, stop=True)
                        osb = opool.tile([128, TQ], F32, tag="osb")
                        nc.vector.tensor_copy(osb[:], ppj[:])
                        nc.scalar.dma_start(
                            out[toff:toff + 128, e * TQ:(e + 1) * TQ],
                            osb[:])

            def emit_qkv(b, tcx, first):
                base = b * T
                off = base + tcx * TQ
                xt = xpool.tile([128, NCT, TQ], F32R, name="xt")
                nc.sync.dma_start(xt[:, 0:NCT // 2],
                                  xT_r[:, 0:NCT // 2, off:off + TQ])
                nc.sync.dma_start(xt[:, NCT // 2:],
                                  xT_r[:, NCT // 2:, off:off + TQ])
                if first:
                    # k/v weight slices + later-phase constants ride behind
                    # the first x chunk; only the q-slice gates the first MM
                    for f in range(1, 3):
                        nc.sync.dma_start(wsb[:, f], wq_r[:, f])
                    nc.sync.dma_start(trisb[:], tri_d[:])
                    nc.sync.dma_start(wpsb[:], wproj[:])
                for f in range(3):
                    pq = ps.tile([128, TQ], F32, tag="s", bufs=5, name="pq")
                    for ct in range(NCT):
                        nc.tensor.matmul(
                            pq[:], wsb[:, f, ct, :], xt[:, ct, :],
                            start=(ct == 0), stop=(ct == NCT - 1),
                        )
                    if f == 0:
                        nc.vector.tensor_scalar_add(
                            qT[:, off:off + TQ], pq[:], bsb[:, 0:1])
                    elif f == 1:
                        nc.vector.tensor_scalar_add(
                            kTt[:, off:off + TQ], pq[:], bsb[:, 1:2])
                    else:
                        vsb = vpool.tile([128, TQ], F32, name="vsb")
                        nc.vector.tensor_scalar_add(
                            vsb[:], pq[:], bsb[:, 2:3])
                        for j in range(TQ // 128):
                            kti = tcx * (TQ // 128) + j
                            pt = ps.tile([128, 128], F32, tag="t", bufs=1,
                                         name="pt")
                            nc.tensor.transpose(
                                pt[:], vsb[:, j * 128:(j + 1) * 128],
                                ident[:])
                            nc.vector.tensor_copy(
                                Vp[:, :, b * NKT + kti, 0:HD],
                                pt[:].rearrange("p (h d) -> p h d", h=HLOC))

            def emit_attn(b, qcx, h):
                nonlocal pend
                base = b * T
                qoff = base + qcx * TQ
                nkt_eff = (TQ // 128) * (qcx + 1)
                hp = HD * h
                # natural k-order; diagonal tiles are the last 4
                diag0 = (TQ // 128) * qcx
                order = list(range(nkt_eff))
                pps = {}
                css = {}
                for kt in order:
                    r = kt - diag0
                    # causal: this k-tile only contributes to columns
                    # >= 128*r; clamp width to >=256 (fp32r full rate)
                    cs = 0 if r < 0 else min(128 * r, TQ - 256)
                    css[kt] = cs
                    s_ps = ps.tile([128, TQ], F32, tag="s", bufs=5,
                                   name="s_ps")
                    nc.tensor.matmul(
                        s_ps[:, cs:TQ],
                        kTt[hp:hp + HD,
                            base + kt * 128:base + (kt + 1) * 128],
                        qT[hp:hp + HD, qoff + cs:qoff + TQ],
                        start=True, stop=True,
                    )
                    pp = ppool.tile([128, TQ], F32R, name="pp")
                    nc.scalar.activation(
                        pp[:, cs:TQ], s_ps[:, cs:TQ], EXP,
                        scale=1.0 / np.sqrt(HD))
                    pps[kt] = pp
                # previous group's norm: recip (DVE) then bcast (PE)
                # then rb (ACT) — its inputs are ready by now
                rb = None
                if pend is not None:
                    rec = spool.tile([1, TQ], F32R, tag="rec", name="rec")
                    with nc.allow_low_precision(
                            reason="f32r reciprocal row"):
                        nc.vector.reciprocal(
                            rec[:], pend["po"][HD:HD + 1, :])
                    pb = ps.tile([HD, TQ], F32, tag="t", bufs=1, name="pb")
                    nc.tensor.matmul(
                        pb[:], onesr[0:1, 0:HD], rec[:],
                        start=True, stop=True)
                    rb = spool.tile([HD, TQ], F32, tag="rb", name="rb")
                    nc.vector.tensor_copy(rb[:], pb[:])
                # this group's diagonal masks: only the mixed 128-col slab
                # needs the triangle; columns left of it are skipped
                # entirely by the cs ranges
                for kt in order:
                    r = kt - diag0
                    if r < 0:
                        continue
                    if 128 * r > css[kt]:
                        # clamped range: zero-left + triangle, 256 wide
                        nc.vector.tensor_mul(
                            pps[kt][:, css[kt]:css[kt] + 256],
                            pps[kt][:, css[kt]:css[kt] + 256],
                            trisb[:])
                    else:
                        sl = 128 * r
                        nc.vector.tensor_mul(
                            pps[kt][:, sl:sl + 128],
                            pps[kt][:, sl:sl + 128],
                            trisb[:, 128:256])
                # previous group's final normalize-multiply into yT
                if pend is not None:
                    nc.vector.tensor_mul(
                        yT[pend["hp"]:pend["hp"] + HD,
                           pend["qoff"]:pend["qoff"] + TQ],
                        pend["po"][0:HD, :], rb[:])
                    proj_ready = pend["last_head"]
                    proj_qoff = pend["qoff"]
                else:
                    proj_ready = False
                # this group's PV accumulation
                po = ps.tile([128, TQ], F32, tag="o", bufs=2, name="po")
                for i, kt in enumerate(order):
                    cs = css[kt]
                    nc.tensor.matmul(
                        po[0:HD + 1, cs:TQ],
                        Vp[:, h, b * NKT + kt, :],
                        pps[kt][:, cs:TQ],
                        start=(i == 0), stop=(i == len(order) - 1),
                    )
                pend = {"po": po, "hp": hp, "qoff": qoff,
                        "last_head": h == HLOC - 1}
                # projection for a completed q-chunk
                if proj_ready:
                    emit_proj(proj_qoff)

            for _rep in range(reps):
                for tcx in range(NQC):
                    emit_qkv(0, tcx, first=(_rep == 0 and tcx == 0))
                for qcx in range(NQC):
                    emit_attn(0, qcx, 0)
                    emit_attn(0, qcx, 1)
                    # batch-1 QKV chunks ride along with batch-0 attention
                    # so ACT (exp) never idles through a QKV-only phase
                    emit_qkv(1, qcx, first=False)
                for qcx in range(NQC):
                    emit_attn(1, qcx, 0)
                    emit_attn(1, qcx, 1)

            # flush the last group's norm + projection
            if pend is not None:
                rec = spool.tile([1, TQ], F32R, tag="rec")
                with nc.allow_low_precision(reason="f32r reciprocal row"):
                    nc.vector.reciprocal(rec[:], pend["po"][HD:HD + 1, :])
                pb = ps.tile([HD, TQ], F32, tag="t", bufs=1)
                nc.tensor.matmul(pb[:], onesr[0:1, 0:HD], rec[:],
                                 start=True, stop=True)
                rb = spool.tile([HD, TQ], F32, tag="rb")
                nc.vector.tensor_copy(rb[:], pb[:])
                nc.vector.tensor_mul(
                    yT[pend["hp"]:pend["hp"] + HD,
                       pend["qoff"]:pend["qoff"] + TQ],
                    pend["po"][0:HD, :], rb[:])
                emit_proj(pend["qoff"])

    nc.finalize()
    return nc


def _make_tri():
    # [zeros | triangle]: tri[p, 128+j] = 1.0 if j >= p; left half all zero.
    # Sliced [:,128:] for exact diagonal slabs; used whole for the clamped
    # (width-256) diagonal tile.
    j = np.arange(128)[None, :]
    p = np.arange(128)[:, None]
    tri = (j >= p).astype(np.float32)
    return np.concatenate([np.zeros((128, 128), np.float32), tri], axis=1)


_NC_CACHE = None
_LAST_IN_MAPS = None


def kernel(x, w_qkv, b_qkv, w_proj, b_proj):
    global _NC_CACHE, _LAST_IN_MAPS
    if _NC_CACHE is None:
        _NC_CACHE = build_nc()
    nc = _NC_CACHE

    x = np.asarray(x, dtype=np.float32)
    w_qkv = np.asarray(w_qkv, dtype=np.float32)
    b_qkv = np.asarray(b_qkv, dtype=np.float32)
    w_proj = np.asarray(w_proj, dtype=np.float32)
    b_proj = np.asarray(b_proj, dtype=np.float32)

    xT = np.ascontiguousarray(x.reshape(BT, D).T)          # [D, BT]
    tri = _make_tri()
    ones_row = np.ones((1, 128), dtype=np.float32)
    ones_col = np.ones((128, 1), dtype=np.float32)

    in_maps = []
    for c in range(N_CORES):
        cols = slice(FQ * c, FQ * (c + 1))
        wq = np.concatenate(
            [w_qkv[:, cols], w_qkv[:, D:][:, cols], w_qkv[:, 2 * D:][:, cols]],
            axis=1)                                        # [D, 384]
        bq = np.stack(
            [b_qkv[cols], b_qkv[D:][cols], b_qkv[2 * D:][cols]],
            axis=1)                                        # [128, 3]
        in_maps.append({
            "xT": xT,
            "wqkv": np.ascontiguousarray(wq),
            "bqkv": np.ascontiguousarray(bq),
            "wproj": np.ascontiguousarray(w_proj[cols, :]),
            "tri": tri,
            "ones_row": ones_row,
            "ones_col": ones_col,
        })

    _LAST_IN_MAPS = in_maps
    res = bass_utils.run_bass_kernel_spmd(
        nc, in_maps, core_ids=list(range(N_CORES)))
    acc = res.results[0]["out"].astype(np.float32).copy()
    for c in range(1, N_CORES):
        acc += res.results[c]["out"]
    acc += b_proj[None, :]
    return acc.reshape(B, T, D)



# revision 28
# speedup vs baseline: 1.2676x; 1.2676x over previous
"""Causal self-attention, tensor-parallel over heads across 8 NeuronCores.

Reference:  qkv = x @ w_qkv + b_qkv;  per-head causal softmax attention;
            out = y @ w_proj + b_proj.
Shapes: x [2, 2048, 1024], H=16 heads, head_dim 64.

Sharding (per core c of 8): heads {2c, 2c+1}.
  - w_qkv columns for q/k/v of those heads  -> [1024, 384]
  - w_proj rows for those heads             -> [128, 1024]
  - each core computes a partial projection output; host sums the 8
    partials (the "all-reduce after proj") and adds b_proj.

On-device strategy (bf16 matmuls, fp32 PSUM):
  - x arrives transposed in bf16 (xT [1024, 4096]); Q/K are computed in
    transposed form [f, t] (weights stationary), the layout scores need.
    V is computed directly in [t, f] layout by flipping the matmul
    (x-chunk stationary, w_v moving) — no transpose anywhere.
  - scores are computed transposed: s_T[k, q] = K Q^T per head; exp on
    ACT (fp32 PSUM in, bf16 out); sum_k exp from a ones-column in V'.
  - causal masking: fully-masked k-tiles skipped, diagonal k-tiles get
    exact column ranges plus a 0/1 triangle multiply (Pool engine).
  - normalization via PE-broadcast reciprocal row (fp32r).
  - attention phases are ACT(exp)-bound: the PE instruction stream is
    software-woven so independent QKV/proj/broadcast matmuls sit between
    exp-gated score/PV matmuls (in-order queues: a waiting instruction
    blocks everything behind it).
  - engine placement: PE matmuls; ACT exp + x loads (DMA queue); DVE
    bias-adds, norm chain, proj eviction, V-transpose DMA issue; Pool
    triangle masks; SP output DMA.
"""

from collections import deque

import numpy as np
import ml_dtypes

import concourse.bacc as bacc
import concourse.mybir as mybir
import concourse.tile as tile
from concourse import bass_utils

# Problem shapes (hardcoded per contest contract)
B, T, D = 2, 2048, 1024
H, HD = 16, 64
N_CORES = 8
HLOC = H // N_CORES      # 2 heads per core
FQ = HLOC * HD           # 128 features per core per q/k/v
BT = B * T               # 4096
TQ = 512                 # q-chunk (matmul moving dim)
KT = 128                 # k-tile (partition dim of transposed scores)
NQC = T // TQ            # 4 q-chunks per batch
NKT = T // KT            # 16 k-tiles per batch
NCT = D // 128           # 8 contraction tiles for qkv

F32 = mybir.dt.float32
F32R = mybir.dt.float32r
BF16 = mybir.dt.bfloat16
EXP = mybir.ActivationFunctionType.Exp


class Fillers:
    """Queue of deferred PE-op closures used to plug exp-gated stalls.
    Closures may spill into the following phase (consumers are always
    >=2 phases after emission), but no further: each phase force-runs
    whatever was already queued when it started."""

    def __init__(self):
        self.q = deque()
        self.popped = 0

    def add(self, fn):
        self.q.append(fn)

    def pop(self, n=1):
        for _ in range(n):
            if not self.q:
                return
            self.popped += 1
            self.q.popleft()()


def build_nc(reps=1):
    nc = bacc.Bacc("TRN2", debug=False)

    xT = nc.dram_tensor("xT", (D, BT), BF16, kind="ExternalInput")
    wqkv = nc.dram_tensor("wqkv", (D, 3 * FQ), BF16, kind="ExternalInput")
    bqkv = nc.dram_tensor("bqkv", (128, 3), F32, kind="ExternalInput")
    wproj = nc.dram_tensor("wproj", (FQ, D), BF16, kind="ExternalInput")
    tri_d = nc.dram_tensor("tri", (128, 128), BF16, kind="ExternalInput")
    ones_d = nc.dram_tensor("ones_row", (1, 128), F32R, kind="ExternalInput")
    bvrow_d = nc.dram_tensor("bv_row", (1, 128), F32R, kind="ExternalInput")
    out = nc.dram_tensor("out", (BT, D), F32, kind="ExternalOutput")

    xT_r = xT.rearrange("(ct p) t -> p ct t", p=128)
    wq_r = wqkv.rearrange("(ct p) (f m) -> p f ct m", p=128, f=3)

    with tile.TileContext(nc) as tc:
        with (
            tc.tile_pool(name="const", bufs=1) as cpool,
            tc.tile_pool(name="xt", bufs=4) as xpool,
            tc.tile_pool(name="pp", bufs=20) as ppool,
            tc.tile_pool(name="sm", bufs=2) as spool,
            tc.tile_pool(name="osb", bufs=6) as opool,
            tc.tile_pool(name="ps", bufs=1, space="PSUM") as ps,
        ):
            # ---- persistent tiles (tiny + first-needed DMAs first) ----
            onesr = cpool.tile([1, 128], F32R)
            nc.sync.dma_start(onesr[:], ones_d[:])
            bsb = cpool.tile([128, 3], F32)
            nc.sync.dma_start(bsb[:], bqkv[:])
            wsb = cpool.tile([128, 3, NCT, 128], BF16)
            nc.sync.dma_start(wsb[:, 0, 0:NCT // 2], wq_r[:, 0, 0:NCT // 2])
            nc.sync.dma_start(wsb[:, 0, NCT // 2:], wq_r[:, 0, NCT // 2:])
            wpsb = cpool.tile([128, D], BF16)
            trisb = cpool.tile([128, 128], BF16)

            qT = cpool.tile([128, BT], BF16)
            kTt = cpool.tile([128, BT], BF16)
            yT = cpool.tile([128, BT], BF16)
            Vp = cpool.tile([128, B * NKT, HLOC, HD + 1], BF16)

            # V' ones-columns written once (data cols rewritten per body,
            # col 64 of each head block never changes)
            nc.gpsimd.memset(Vp[:, :, :, HD:HD + 1], 1.0)

            # b_v broadcast tile [t, f] (bias varies along the free axis
            # in V's layout, so tensor_scalar can't supply it): one-time
            # PE broadcast of the host-sent row, evicted to SBUF
            bvb = cpool.tile([128, 128], BF16)
            bvr = cpool.tile([1, 128], F32R)
            nc.sync.dma_start(bvr[:], bvrow_d[:])
            pbv = ps.tile([128, 128], F32, tag="t", bufs=1, name="pbv")
            nc.tensor.matmul(pbv[:], onesr[:], bvr[:], start=True, stop=True)
            nc.vector.tensor_copy(bvb[:], pbv[:])

            state = {"pend": None, "proj": None}

            def qkv_closures(b, tcx, first):
                """x loads now; returns PE-matmul closures for this chunk."""
                base = b * T
                off = base + tcx * TQ
                xt = xpool.tile([128, NCT, TQ], BF16, name="xt")
                for q4 in range(4):
                    nc.scalar.dma_start(
                        xt[:, 2 * q4:2 * (q4 + 1)],
                        xT_r[:, 2 * q4:2 * (q4 + 1), off:off + TQ])
                if first:
                    for f in range(1, 3):
                        nc.sync.dma_start(wsb[:, f], wq_r[:, f])
                    nc.sync.dma_start(trisb[:], tri_d[:])
                    nc.sync.dma_start(wpsb[:], wproj[:])

                hold = {}

                def mm(f, ct):
                    if ct == 0:
                        hold["pq"] = ps.tile([128, TQ], F32, tag="q",
                                             bufs=1, name="pq")
                    pq = hold["pq"]
                    nc.tensor.matmul(
                        pq[:], wsb[:, f, ct, :], xt[:, ct, :],
                        start=(ct == 0), stop=(ct == NCT - 1))
                    if ct != NCT - 1:
                        return
                    if f == 0:
                        nc.vector.tensor_scalar_add(
                            qT[:, off:off + TQ], pq[:], bsb[:, 0:1])
                    else:
                        nc.vector.tensor_scalar_add(
                            kTt[:, off:off + TQ], pq[:], bsb[:, 1:2])

                def vmm(j, ct):
                    # V in [t, f] layout: x-chunk stationary, w_v moving
                    if ct == 0:
                        hold["pv"] = ps.tile([128, 128], F32, tag="q",
                                             bufs=1, name="pv")
                    pv = hold["pv"]
                    nc.tensor.matmul(
                        pv[:], xt[:, ct, j * 128:(j + 1) * 128],
                        wsb[:, 2, ct, :],
                        start=(ct == 0), stop=(ct == NCT - 1))
                    if ct != NCT - 1:
                        return
                    kti = b * NKT + tcx * (TQ // 128) + j
                    nc.vector.tensor_add(
                        Vp[:, kti, :, 0:HD],
                        pv[:].rearrange("p (h d) -> p h d", h=HLOC),
                        bvb[:].rearrange("p (h d) -> p h d", h=HLOC))

                return (
                    [(lambda f=f, ct=ct: mm(f, ct))
                     for f in range(2) for ct in range(NCT)]
                    + [(lambda j=j, ct=ct: vmm(j, ct))
                       for j in range(TQ // 128) for ct in range(NCT)]
                )

            def norm_closure(pend):
                """Returns (rec_fill, pb_fill): reciprocal row on DVE, then
                PE-broadcast + normalize-multiply straight out of PSUM."""
                hold = {}

                def rec_fill():
                    rec = spool.tile([1, TQ], F32R, tag="rec", name="rec")
                    with nc.allow_low_precision(
                            reason="f32r reciprocal row"):
                        nc.vector.reciprocal(rec[:], pend["po"][HD:HD + 1, :])
                    hold["rec"] = rec

                def pb_fill():
                    pb = ps.tile([HD, TQ], F32, tag="t", bufs=1, name="pb")
                    nc.tensor.matmul(pb[:], onesr[0:1, 0:HD],
                                     hold["rec"][:], start=True, stop=True)
                    # TensorTensor may read only one PSUM operand: stage rb
                    rb = spool.tile([HD, TQ], F32, tag="rb", name="rb")
                    nc.vector.tensor_copy(rb[:], pb[:])
                    nc.vector.tensor_mul(
                        yT[pend["hp"]:pend["hp"] + HD,
                           pend["qoff"]:pend["qoff"] + TQ],
                        pend["po"][0:HD, :], rb[:])

                return rec_fill, pb_fill

            def proj_closures(qoff_abs):
                def tt_fill(tt):
                    toff = qoff_abs + tt * 128
                    osb = opool.tile([128, D], F32, tag="osb", name="osb")
                    for e in range(D // TQ):
                        ppj = ps.tile([128, TQ], F32, tag="t", bufs=1,
                                      name="ppj")
                        nc.tensor.matmul(
                            ppj[:], yT[:, toff:toff + 128],
                            wpsb[:, e * TQ:(e + 1) * TQ],
                            start=True, stop=True)
                        nc.vector.tensor_copy(
                            osb[:, e * TQ:(e + 1) * TQ], ppj[:])
                    nc.sync.dma_start(out[toff:toff + 128, :], osb[:])

                return [lambda tt=tt: tt_fill(tt) for tt in range(TQ // 128)]

            def attn_group(b, qcx, h, fill):
                """Scores+exp+mask then PV for one (batch, q-chunk, head),
                weaving fillers into the exp-gated sections."""
                base = b * T
                qoff = base + qcx * TQ
                nkt_eff = (TQ // 128) * (qcx + 1)
                hp = HD * h
                diag0 = (TQ // 128) * qcx
                pps = {}
                css = {}
                for kt in range(nkt_eff):
                    r = kt - diag0
                    cs = 0 if r < 0 else 128 * r
                    css[kt] = cs
                    s_ps = ps.tile([128, TQ], F32, tag="s", bufs=4,
                                   name="s_ps")
                    nc.tensor.matmul(
                        s_ps[:, cs:TQ],
                        kTt[hp:hp + HD,
                            base + kt * 128:base + (kt + 1) * 128],
                        qT[hp:hp + HD, qoff + cs:qoff + TQ],
                        start=True, stop=True,
                    )
                    pp = ppool.tile([128, TQ], BF16, name="pp")
                    nc.scalar.activation(
                        pp[:, cs:TQ], s_ps[:, cs:TQ], EXP,
                        scale=1.0 / np.sqrt(HD))
                    if r >= 0:
                        nc.gpsimd.tensor_mul(
                            pp[:, cs:cs + 128], pp[:, cs:cs + 128],
                            trisb[:])
                    pps[kt] = pp
                    if kt >= 3:
                        fill.pop(2)
                po = ps.tile([128, TQ], F32, tag="o", bufs=2, name="po")
                for kt in range(nkt_eff):
                    cs = css[kt]
                    nc.tensor.matmul(
                        po[0:HD + 1, cs:TQ],
                        Vp[:, b * NKT + kt, h, :],
                        pps[kt][:, cs:TQ],
                        start=(kt == 0), stop=(kt == nkt_eff - 1),
                    )
                    fill.pop(2)
                state["pend"] = {"po": po, "hp": hp, "qoff": qoff,
                                 "last_head": h == HLOC - 1}

            fill = Fillers()

            def phase(b, qcx, ride):
                """attention for (b,qcx) both heads; rides one qkv chunk
                and the previous chunk's norm tail + projection."""
                must_run = len(fill.q)
                fill.popped = 0
                if ride is not None:
                    for c in qkv_closures(*ride):
                        fill.add(c)
                for h in range(HLOC):
                    pend = state["pend"]
                    if pend is not None:
                        rec_fill, pb_fill = norm_closure(pend)
                        fill.q.appendleft(rec_fill)
                        fill.q.insert(min(4, len(fill.q)), pb_fill)
                        must_run += 2
                        if pend["last_head"]:
                            projs = proj_closures(pend["qoff"])
                            for i, c in enumerate(projs):
                                fill.q.insert(
                                    min(6 + 2 * i, len(fill.q)), c)
                            must_run += len(projs)
                    state["pend"] = None
                    attn_group(b, qcx, h, fill)
                fill.pop(max(0, must_run - fill.popped))

            # prologue: batch-0 QKV, unwoven
            for tcx in range(NQC):
                for c in qkv_closures(0, tcx, first=(tcx == 0)):
                    c()
            for _rep in range(reps):
                for qcx in range(NQC):
                    phase(0, qcx, ride=(1, qcx, False))
                for qcx in range(NQC):
                    ride = (0, qcx, False) if _rep < reps - 1 else None
                    phase(1, qcx, ride)

            # flush the last group's norm + projection
            fill.pop(len(fill.q))
            pend = state["pend"]
            rec_fill, pb_fill = norm_closure(pend)
            rec_fill()
            pb_fill()
            for c in proj_closures(pend["qoff"]):
                c()

    nc.finalize()
    return nc


def _make_tri():
    # tri[p, j] = 1.0 if j >= p (upper triangle incl. diagonal)
    j = np.arange(128)[None, :]
    p = np.arange(128)[:, None]
    return (j >= p).astype(ml_dtypes.bfloat16)


_NC_CACHE = None
_LAST_IN_MAPS = None


def kernel(x, w_qkv, b_qkv, w_proj, b_proj):
    global _NC_CACHE, _LAST_IN_MAPS
    if _NC_CACHE is None:
        _NC_CACHE = build_nc()
    nc = _NC_CACHE

    x = np.asarray(x, dtype=np.float32)
    w_qkv = np.asarray(w_qkv, dtype=np.float32)
    b_qkv = np.asarray(b_qkv, dtype=np.float32)
    w_proj = np.asarray(w_proj, dtype=np.float32)
    b_proj = np.asarray(b_proj, dtype=np.float32)

    xT = np.ascontiguousarray(
        x.reshape(BT, D).T).astype(ml_dtypes.bfloat16)     # [D, BT]
    tri = _make_tri()
    ones_row = np.ones((1, 128), dtype=np.float32)

    in_maps = []
    for c in range(N_CORES):
        cols = slice(FQ * c, FQ * (c + 1))
        wq = np.concatenate(
            [w_qkv[:, cols], w_qkv[:, D:][:, cols], w_qkv[:, 2 * D:][:, cols]],
            axis=1).astype(ml_dtypes.bfloat16)             # [D, 384]
        bq = np.stack(
            [b_qkv[cols], b_qkv[D:][cols], b_qkv[2 * D:][cols]],
            axis=1)                                        # [128, 3]
        in_maps.append({
            "xT": xT,
            "wqkv": np.ascontiguousarray(wq),
            "bqkv": np.ascontiguousarray(bq),
            "wproj": np.ascontiguousarray(
                w_proj[cols, :].astype(ml_dtypes.bfloat16)),
            "tri": tri,
            "ones_row": ones_row,
            "bv_row": np.ascontiguousarray(
                b_qkv[2 * D:][cols][None, :].astype(np.float32)),
        })

    _LAST_IN_MAPS = in_maps
    res = bass_utils.run_bass_kernel_spmd(
        nc, in_maps, core_ids=list(range(N_CORES)))
    acc = res.results[0]["out"].astype(np.float32).copy()
    for c in range(1, N_CORES):
        acc += res.results[c]["out"]
    acc += b_proj[None, :]
    return acc.reshape(B, T, D)
